# revision 17
# baseline (speedup 1.0000x reference)
"""DarkChannelPriorLoss Trainium2 kernel.

reference semantics: dcp = |maxpool3d(-rgb, kernel=(3,41,41), stride=1,
pad=(0,20,20), -inf)| which for rgb >= 0 equals the separable min-pool
of rgb (channel-min, then clamped 41-wide sliding mins along W and H);
loss = mean(dcp).  Output: (loss, dcp[B,1,512,512]).

Distribution: pure data parallel, batch 32 -> 8 cores x 4 images.

Per-core pipeline (shapes hardcoded), chunked at 128x512 granularity for
fine-grained overlap of DMA / DVE / PE / ACT:
  1. HWDGE loads (2 per image, SP ring); channel-min on DVE
  2. W-axis clamped sliding min (window 41) per row-chunk via van Herk:
     two segmented min-scans (tensor_tensor_scan; 41-block resets via a
     bias-add array, suffix scan through reversed APs) + combine + edges
  3. PE transpose (identity matmul) + ACT PSUM->SBUF copies
  4. H-axis sliding min per col-chunk (same van Herk, transposed layout)
  5. PE transpose back, store on the ACT HWDGE ring
  6. loss partials: ones-matmul partition sums accumulated in one PSUM bank
"""
import numpy as np

import concourse.bass as bass
import concourse.tile as tile
from concourse import bacc, mybir
from concourse.bass_utils import run_bass_kernel_spmd
from contextlib import ExitStack

F32 = mybir.dt.float32
P = 128
L = 512
NB = 4            # 512 rows = 4 x 128 partitions
F = NB * L        # packed free dim
WIN = 41
PAD = 20
BIG = 1e30
N_CORES = 8
N_IMG = 4         # images per core
B = N_CORES * N_IMG
MIN = mybir.AluOpType.min
ADD = mybir.AluOpType.add


def _build_nc(n_img: int = N_IMG, reps: int = 1, span: int = 1,
              i8bias: bool = False):
    nc = bacc.Bacc(None, target_bir_lowering=False, debug=False)

    rgb = nc.dram_tensor("rgb", [n_img, 3, L, L], F32, kind="ExternalInput")
    dcp = nc.dram_tensor("dcp", [n_img, 1, L, L], F32, kind="ExternalOutput")
    partial = nc.dram_tensor("partial", [1, 1], F32, kind="ExternalOutput")

    ident_d = nc.inline_tensor(np.eye(P, dtype=np.float32), name="ident_const")
    ones_d = nc.inline_tensor(np.ones((P, 1), np.float32), name="ones_const")

    SPAN = span
    with tile.TileContext(nc) as tc, ExitStack() as ctx:
        cpool = ctx.enter_context(tc.tile_pool(name="consts", bufs=1))
        apool = ctx.enter_context(tc.tile_pool(name="a", bufs=3))
        bpool = ctx.enter_context(tc.tile_pool(name="b", bufs=2))
        spool = ctx.enter_context(tc.tile_pool(name="scans", bufs=max(2, 6 // SPAN)))
        tpool = ctx.enter_context(tc.tile_pool(name="t", bufs=2))
        dpool = ctx.enter_context(tc.tile_pool(name="d", bufs=2))
        pspool = ctx.enter_context(tc.tile_pool(name="ps", bufs=4, space="PSUM"))
        lpool = ctx.enter_context(tc.tile_pool(name="loss", bufs=1, space="PSUM"))

        # bias: reset-add array, nonzero at 41-block starts in each 512-run
        # (+ sentinel so the reversed/shifted view resets at every block
        # end); built on-chip.  int8 variant: data values are < 1, so +127
        # is enough to dominate the min -- and the narrow stream halves the
        # scan's read traffic.
        bdt = mybir.dt.int8 if i8bias else F32
        bval = 127 if i8bias else BIG
        bias = cpool.tile([P, F + 1], bdt)
        ident = cpool.tile([P, P], F32)
        ones = cpool.tile([P, 1], F32)
        nc.gpsimd.memset(bias[:], 0)
        bv = bias[:, 0:F].rearrange("p (r w) -> p r w", r=NB)[:, :, 0:L:WIN]
        nc.gpsimd.memset(bv, bval)
        nc.gpsimd.memset(bias[:, F:F + 1], bval)
        nc.sync.dma_start(ident[:], ident_d[:])
        nc.sync.dma_start(ones[:], ones_d[:])

        loss_ps = lpool.tile([1, L], F32)

        rep_ctx = tc.For_i(0, reps, 1) if reps > 1 else None
        if rep_ctx is not None:
            rep_ctx.__enter__()

        def sliding_min_span(src, k0, nr):
            """window-41 clamped sliding min on 512-chunks [k0, k0+nr) of a
            [P,F] tile, as one set of ops with 3D views; result overwrites
            the src chunks (combine reads scratch pref/suf tiles)."""
            s0 = k0 * L
            n = nr * L
            sc = src[:, s0:s0 + n]
            pref = spool.tile([P, SPAN * L], F32, tag="pref")
            suf = spool.tile([P, SPAN * L], F32, tag="suf")
            pc = pref[:, 0:n]
            uc = suf[:, 0:n]
            nc.vector.tensor_tensor_scan(
                pc, bias[:, 0:n], sc, BIG, op0=ADD, op1=MIN)
            nc.vector.tensor_tensor_scan(
                uc[:, n - 1::-1], bias[:, n:0:-1], sc[:, n - 1::-1], BIG,
                op0=ADD, op1=MIN)
            s3 = sc.rearrange("p (r w) -> p r w", r=nr)
            p3 = pc.rearrange("p (r w) -> p r w", r=nr)
            u3 = uc.rearrange("p (r w) -> p r w", r=nr)
            # out[i] = min(S[i-20], P[i+20]) for i in [20, 492)
            nc.vector.tensor_tensor(
                s3[:, :, PAD:L - PAD], u3[:, :, 0:L - 2 * PAD],
                p3[:, :, 2 * PAD:L], op=MIN)
            # out[i] = P[i+20] for i in [0, 20)
            nc.scalar.copy(s3[:, :, 0:PAD], p3[:, :, PAD:2 * PAD])
            # out[i] = min(S[i-20], P[511]) for i in [492, 512)
            nc.vector.tensor_tensor(
                s3[:, :, L - PAD:L], u3[:, :, L - 2 * PAD:L - PAD],
                p3[:, :, L - 1:L].broadcast_to((P, nr, PAD)), op=MIN)

        for i in range(n_img):
            # loads: one 1MB HWDGE DMA per channel on the SP ring
            CH = apool.tile([P, 3 * F], F32, tag="CH")
            chv = CH[:].rearrange("p (c r w) -> p c r w", c=3, r=NB)
            for c in range(3):
                nc.sync.dma_start(
                    chv[:, c], rgb[i, c].rearrange("(hb p) w -> p hb w", p=P))

            # W-stage per row-span
            A = bpool.tile([P, F], F32, tag="A")
            for hb in range(0, NB, SPAN):
                s0 = hb * L
                n = SPAN * L
                nc.vector.tensor_tensor(
                    A[:, s0:s0 + n], CH[:, s0:s0 + n], CH[:, F + s0:F + s0 + n],
                    op=MIN)
                nc.vector.tensor_tensor(
                    A[:, s0:s0 + n], A[:, s0:s0 + n],
                    CH[:, 2 * F + s0:2 * F + s0 + n], op=MIN)
                sliding_min_span(A, hb, SPAN)

            # per col-chunk: transpose W -> T, H-stage, loss matmul
            T = tpool.tile([P, F], F32, tag="T")
            for wb in range(NB):
                pt = pspool.tile([P, L], F32, tag="pt")
                for hb in range(NB):
                    nc.tensor.transpose(
                        pt[:, hb * P:(hb + 1) * P],
                        A[:, hb * L + wb * P: hb * L + wb * P + P],
                        ident[:])
                nc.scalar.copy(T[:, wb * L:(wb + 1) * L], pt[:])
                if (wb + 1) % SPAN == 0:
                    sliding_min_span(T, wb + 1 - SPAN, SPAN)
                    for wk in range(wb + 1 - SPAN, wb + 1):
                        nc.tensor.matmul(
                            loss_ps[:], ones[:], T[:, wk * L:(wk + 1) * L],
                            start=(i == 0 and wk == 0),
                            stop=(i == n_img - 1 and wk == NB - 1))

            # transpose back T -> D (natural layout), store on ACT ring
            D = dpool.tile([P, F], F32, tag="D")
            for hb in range(NB):
                pt2 = pspool.tile([P, L], F32, tag="pt")
                for wb in range(NB):
                    nc.tensor.transpose(
                        pt2[:, wb * P:(wb + 1) * P],
                        T[:, wb * L + hb * P: wb * L + hb * P + P],
                        ident[:])
                nc.scalar.copy(D[:, hb * L:(hb + 1) * L], pt2[:])

            nc.scalar.dma_start(
                dcp[i, 0].rearrange("(hb p) w -> p hb w", p=P),
                D[:].rearrange("p (r w) -> p r w", r=NB))

        if rep_ctx is not None:
            rep_ctx.__exit__(None, None, None)

        lt = dpool.tile([1, L], F32, tag="lt")
        nc.vector.tensor_copy(lt[:], loss_ps[:])
        lsum = dpool.tile([1, 1], F32, tag="lsum")
        nc.vector.tensor_reduce(lsum[:], lt[:], axis=mybir.AxisListType.X,
                                op=ADD)
        nc.sync.dma_start(partial[:], lsum[:])

    nc.compile()
    return nc


_NC_CACHE = None


def _get_nc():
    global _NC_CACHE
    if _NC_CACHE is None:
        _NC_CACHE = _build_nc()
    return _NC_CACHE


def run_sharded(rgb: np.ndarray, trace: bool = False):
    """Run the SPMD kernel; returns (loss, dcp, BassKernelResults)."""
    assert rgb.shape == (B, 3, L, L), rgb.shape
    rgb = np.ascontiguousarray(rgb, dtype=np.float32)
    nc = _get_nc()
    in_maps = [{"rgb": rgb[i * N_IMG:(i + 1) * N_IMG]} for i in range(N_CORES)]
    res = run_bass_kernel_spmd(nc, in_maps, list(range(N_CORES)), trace=trace)
    dcp = np.concatenate([r["dcp"] for r in res.results], axis=0)
    total = np.sum([r["partial"][0, 0] for r in res.results], dtype=np.float64)
    loss = np.float32(total / (B * L * L))
    return loss, dcp, res


def kernel(rgb: np.ndarray):
    loss, dcp, _ = run_sharded(np.asarray(rgb))
    return loss, dcp


# revision 18
# speedup vs baseline: 1.0793x; 1.0793x over previous
"""DarkChannelPriorLoss Trainium2 kernel.

reference semantics: dcp = |maxpool3d(-rgb, kernel=(3,41,41), stride=1,
pad=(0,20,20), -inf)| which for rgb >= 0 equals the separable min-pool
of rgb (channel-min, then clamped 41-wide sliding mins along W and H);
loss = mean(dcp).  Output: (loss, dcp[B,1,512,512]).

Distribution: pure data parallel, batch 32 -> 8 cores x 4 images.

Per-core pipeline (shapes hardcoded), chunked at 128x512 granularity for
fine-grained overlap of DMA / DVE / PE / ACT:
  1. HWDGE loads (2 per image, SP ring); channel-min on DVE
  2. W-axis clamped sliding min (window 41) per row-chunk via van Herk:
     two segmented min-scans (tensor_tensor_scan; 41-block resets via a
     bias-add array, suffix scan through reversed APs) + combine + edges
  3. PE transpose (identity matmul) + ACT PSUM->SBUF copies
  4. H-axis sliding min per col-chunk (same van Herk, transposed layout)
  5. PE transpose back, store on the ACT HWDGE ring
  6. loss partials: ones-matmul partition sums accumulated in one PSUM bank
"""
import numpy as np

import concourse.bass as bass
import concourse.tile as tile
from concourse import bacc, mybir
from concourse.bass_utils import run_bass_kernel_spmd
from contextlib import ExitStack

F32 = mybir.dt.float32
P = 128
L = 512
NB = 4            # 512 rows = 4 x 128 partitions
F = NB * L        # packed free dim
WIN = 41
PAD = 20
BIG = 1e30
N_CORES = 8
N_IMG = 4         # images per core
B = N_CORES * N_IMG
MIN = mybir.AluOpType.min
ADD = mybir.AluOpType.add


def _build_nc(n_img: int = N_IMG, reps: int = 1, span: int = 1,
              i8bias: bool = True):
    nc = bacc.Bacc(None, target_bir_lowering=False, debug=False)

    rgb = nc.dram_tensor("rgb", [n_img, 3, L, L], F32, kind="ExternalInput")
    dcp = nc.dram_tensor("dcp", [n_img, 1, L, L], F32, kind="ExternalOutput")
    partial = nc.dram_tensor("partial", [1, 1], F32, kind="ExternalOutput")

    ident_d = nc.inline_tensor(np.eye(P, dtype=np.float32), name="ident_const")
    ones_d = nc.inline_tensor(np.ones((P, 1), np.float32), name="ones_const")

    SPAN = span
    with tile.TileContext(nc) as tc, ExitStack() as ctx:
        cpool = ctx.enter_context(tc.tile_pool(name="consts", bufs=1))
        apool = ctx.enter_context(tc.tile_pool(name="a", bufs=3))
        bpool = ctx.enter_context(tc.tile_pool(name="b", bufs=2))
        spool = ctx.enter_context(tc.tile_pool(name="scans", bufs=max(2, 6 // SPAN)))
        tpool = ctx.enter_context(tc.tile_pool(name="t", bufs=2))
        dpool = ctx.enter_context(tc.tile_pool(name="d", bufs=2))
        pspool = ctx.enter_context(tc.tile_pool(name="ps", bufs=4, space="PSUM"))
        lpool = ctx.enter_context(tc.tile_pool(name="loss", bufs=1, space="PSUM"))

        # bias: reset-add array, nonzero at 41-block starts in each 512-run
        # (+ sentinel so the reversed/shifted view resets at every block
        # end); built on-chip.  int8 variant: data values are < 1, so +127
        # is enough to dominate the min -- and the narrow stream halves the
        # scan's read traffic.
        bdt = mybir.dt.int8 if i8bias else F32
        bval = 127 if i8bias else BIG
        bias = cpool.tile([P, F + 1], bdt)
        ident = cpool.tile([P, P], F32)
        ones = cpool.tile([P, 1], F32)
        nc.gpsimd.memset(bias[:], 0)
        bv = bias[:, 0:F].rearrange("p (r w) -> p r w", r=NB)[:, :, 0:L:WIN]
        nc.gpsimd.memset(bv, bval)
        nc.gpsimd.memset(bias[:, F:F + 1], bval)
        nc.sync.dma_start(ident[:], ident_d[:])
        nc.sync.dma_start(ones[:], ones_d[:])

        loss_ps = lpool.tile([1, L], F32)

        rep_ctx = tc.For_i(0, reps, 1) if reps > 1 else None
        if rep_ctx is not None:
            rep_ctx.__enter__()

        def sliding_min_span(src, k0, nr):
            """window-41 clamped sliding min on 512-chunks [k0, k0+nr) of a
            [P,F] tile, as one set of ops with 3D views; result overwrites
            the src chunks (combine reads scratch pref/suf tiles)."""
            s0 = k0 * L
            n = nr * L
            sc = src[:, s0:s0 + n]
            pref = spool.tile([P, SPAN * L], F32, tag="pref")
            suf = spool.tile([P, SPAN * L], F32, tag="suf")
            pc = pref[:, 0:n]
            uc = suf[:, 0:n]
            nc.vector.tensor_tensor_scan(
                pc, bias[:, 0:n], sc, BIG, op0=ADD, op1=MIN)
            nc.vector.tensor_tensor_scan(
                uc[:, n - 1::-1], bias[:, n:0:-1], sc[:, n - 1::-1], BIG,
                op0=ADD, op1=MIN)
            s3 = sc.rearrange("p (r w) -> p r w", r=nr)
            p3 = pc.rearrange("p (r w) -> p r w", r=nr)
            u3 = uc.rearrange("p (r w) -> p r w", r=nr)
            # out[i] = min(S[i-20], P[i+20]) for i in [20, 492)
            nc.vector.tensor_tensor(
                s3[:, :, PAD:L - PAD], u3[:, :, 0:L - 2 * PAD],
                p3[:, :, 2 * PAD:L], op=MIN)
            # out[i] = P[i+20] for i in [0, 20)
            nc.scalar.copy(s3[:, :, 0:PAD], p3[:, :, PAD:2 * PAD])
            # out[i] = min(S[i-20], P[511]) for i in [492, 512)
            nc.vector.tensor_tensor(
                s3[:, :, L - PAD:L], u3[:, :, L - 2 * PAD:L - PAD],
                p3[:, :, L - 1:L].broadcast_to((P, nr, PAD)), op=MIN)

        for i in range(n_img):
            # loads: one 1MB HWDGE DMA per channel on the SP ring
            CH = apool.tile([P, 3 * F], F32, tag="CH")
            chv = CH[:].rearrange("p (c r w) -> p c r w", c=3, r=NB)
            for c in range(3):
                nc.sync.dma_start(
                    chv[:, c], rgb[i, c].rearrange("(hb p) w -> p hb w", p=P))

            # W-stage per row-span
            A = bpool.tile([P, F], F32, tag="A")
            for hb in range(0, NB, SPAN):
                s0 = hb * L
                n = SPAN * L
                nc.vector.tensor_tensor(
                    A[:, s0:s0 + n], CH[:, s0:s0 + n], CH[:, F + s0:F + s0 + n],
                    op=MIN)
                nc.vector.tensor_tensor(
                    A[:, s0:s0 + n], A[:, s0:s0 + n],
                    CH[:, 2 * F + s0:2 * F + s0 + n], op=MIN)
                sliding_min_span(A, hb, SPAN)

            # per col-chunk: transpose W -> T, H-stage, loss matmul
            T = tpool.tile([P, F], F32, tag="T")
            for wb in range(NB):
                pt = pspool.tile([P, L], F32, tag="pt")
                for hb in range(NB):
                    nc.tensor.transpose(
                        pt[:, hb * P:(hb + 1) * P],
                        A[:, hb * L + wb * P: hb * L + wb * P + P],
                        ident[:])
                nc.scalar.copy(T[:, wb * L:(wb + 1) * L], pt[:])
                if (wb + 1) % SPAN == 0:
                    sliding_min_span(T, wb + 1 - SPAN, SPAN)
                    for wk in range(wb + 1 - SPAN, wb + 1):
                        nc.tensor.matmul(
                            loss_ps[:], ones[:], T[:, wk * L:(wk + 1) * L],
                            start=(i == 0 and wk == 0),
                            stop=(i == n_img - 1 and wk == NB - 1))

            # transpose back T -> D (natural layout), store on ACT ring
            D = dpool.tile([P, F], F32, tag="D")
            for hb in range(NB):
                pt2 = pspool.tile([P, L], F32, tag="pt")
                for wb in range(NB):
                    nc.tensor.transpose(
                        pt2[:, wb * P:(wb + 1) * P],
                        T[:, wb * L + hb * P: wb * L + hb * P + P],
                        ident[:])
                nc.scalar.copy(D[:, hb * L:(hb + 1) * L], pt2[:])

            nc.scalar.dma_start(
                dcp[i, 0].rearrange("(hb p) w -> p hb w", p=P),
                D[:].rearrange("p (r w) -> p r w", r=NB))

        if rep_ctx is not None:
            rep_ctx.__exit__(None, None, None)

        lt = dpool.tile([1, L], F32, tag="lt")
        nc.vector.tensor_copy(lt[:], loss_ps[:])
        lsum = dpool.tile([1, 1], F32, tag="lsum")
        nc.vector.tensor_reduce(lsum[:], lt[:], axis=mybir.AxisListType.X,
                                op=ADD)
        nc.sync.dma_start(partial[:], lsum[:])

    nc.compile()
    return nc


_NC_CACHE = None


def _get_nc():
    global _NC_CACHE
    if _NC_CACHE is None:
        _NC_CACHE = _build_nc()
    return _NC_CACHE


def run_sharded(rgb: np.ndarray, trace: bool = False):
    """Run the SPMD kernel; returns (loss, dcp, BassKernelResults)."""
    assert rgb.shape == (B, 3, L, L), rgb.shape
    rgb = np.ascontiguousarray(rgb, dtype=np.float32)
    nc = _get_nc()
    in_maps = [{"rgb": rgb[i * N_IMG:(i + 1) * N_IMG]} for i in range(N_CORES)]
    res = run_bass_kernel_spmd(nc, in_maps, list(range(N_CORES)), trace=trace)
    dcp = np.concatenate([r["dcp"] for r in res.results], axis=0)
    total = np.sum([r["partial"][0, 0] for r in res.results], dtype=np.float64)
    loss = np.float32(total / (B * L * L))
    return loss, dcp, res


def kernel(rgb: np.ndarray):
    loss, dcp, _ = run_sharded(np.asarray(rgb))
    return loss, dcp
